# revision 62
# baseline (speedup 1.0000x reference)
"""Trainium2 Bass kernel for additive-attention pooling.

Reference math (per sample b):
    score  = tanh(x @ W_w + W_b)          # [T, U]
    logits = score @ V_w + V_b            # [T, 1]
    attn   = softmax(logits, axis=T)
    out    = sum_t attn[t] * x[t, :]      # [D]

V_b cancels in the softmax and is dropped. Softmax max-subtraction is
skipped: logits stay in [-5, 5] for this input scale, safe in bf16 exp.

Sharding: data-parallel over batch, 8 samples per core on 8 NeuronCores.

Precision strategy (simulated end-to-end rel err 1.42e-2 vs gate 2e-2):
  - GEMM path: x ships as int8 (x*22 rounded); SWDGE (gpsimd) DMAs cast
    int8 -> bf16 in flight, landing exact small integers in SBUF. int8's
    uniform absolute quantization error beats fp8-e4m3's relative error
    ~3x for N(0,1) data, and W (the error-critical operand: its error is
    systematic across t and does not average out) stays bf16, pre-divided
    by 22 host-side so no descale is needed.
  - wsum path: xn ships as fp8-e4m3 (HWDGE, no cast) and the exp weights
    are fp8, enabling DoubleRow matmuls (2 fp8 MACs/cell) that halve the
    weighted-sum TensorE time.

Layouts (per core, S=8 samples):
  xT_i8 [S, 128, 2, T]        d = dc*128 + ki, score GEMM (d on part.)
  xn_f8 [S, 128, 16, 2, 272]  t = cp*256 + ko*128 + p; col 256 = 1.0
                              (denominator), cols 257.. pad so the
                              DoubleRow ko-stride is 16-byte aligned
  w     [128, 2, 256]         bf16 W/22, d = dc*128 + ki
  wb/v  [128, 2]              f32 per-u bias / V weights

Pipeline per group g (1024 t's, 32 groups), software-pipelined via Tile:
  1. GEMM (TensorE): per uc: 4 matmuls (2 dc x 2 halves) N=512 into a
     [128, 1024] psum tile (2 banks), W blocks stationary.
  2. tanh (ScalarE): one [128, 1024] ACT per uc, psum -> bf16 SBUF,
     bias = W_b chunk (batched ACTs amortize the 352-cyc fixed cost).
  3. V-fold (VectorE): z = V0*tanh_u0 + V1*tanh_u1, [128, 1024] ops.
  4. V-dot (TensorE): per 128-t chunk, lhsT = z chunk, rhs = ones ->
     logit col [t,1] at lg3[:, cc%2, cc//2] (the DoubleRow pairing);
     interleaved between GEMM matmuls so LDWEIGHTS stay hidden.
  5. exp (ScalarE): one [128, 2, 4] ACT per group -> fp8 weights
     (enables LAG_W=3 and a short drain tail).
  6. wsum (TensorE): DoubleRow, lhsT = fp8 weight pair [128, 2, 1],
     rhs = xn chunk-pair [128, 2, 272], accumulated over the sample in a
     single partition-0 psum row (DoubleRow outputs must start at
     partition 0, and an accumulating bank cannot be shared: start=True
     wipes the whole bank's has_written bits).
  7. finalize (VectorE): copy num|den psum row -> SBUF, DMA out; the
     division happens on the host after the gather.

HAM management: a burst of dependency-free warmup matmuls at kernel
start (during the DMA fill) and at the head of the drain keeps the PE
clock at 2.4 GHz; the last group runs tanh/fold/exp at half granularity
to shorten the serial drain chain.

Measured on 8 trn2 cores: ~111 us (baseline bf16 kernel: ~141 us).
"""

import numpy as np
import ml_dtypes

# ---- problem constants (hardcoded; kernel.py must be self-contained) ----
B, T, D, U = 64, 4096, 256, 256
N_CORES = 8
S = B // N_CORES          # samples per core
TT = 512                  # t-tile
GT = 1024                 # t's per pipeline group (2 tiles)
N_GROUPS = T // GT        # groups per sample (4)
NG = S * N_GROUPS         # total pipeline groups (32)
CH = GT // 128            # 128-row chunks per group (8)
NCH = T // 128            # chunks per sample (32)
XS = 22.0                 # int8 quantization scale for x
DP = 272                  # xn free size: D padded to a 16-byte multiple + den
LAG_L2 = 1                # V-dot lag in groups
LAG_W = 3                 # weighted-sum lag in groups (per-group exps)

BF16 = ml_dtypes.bfloat16
FP8 = ml_dtypes.float8_e4m3

_CACHE = {}


def _build():
    import concourse.bass as bass
    import concourse.tile as tile
    from concourse import bacc, mybir
    from concourse.bass import ds, ts

    f32 = mybir.dt.float32
    bf16 = mybir.dt.bfloat16
    i8 = mybir.dt.int8
    f8 = mybir.dt.float8e4
    DR = mybir.MatmulPerfMode.DoubleRow
    Tanh = mybir.ActivationFunctionType.Tanh
    Exp = mybir.ActivationFunctionType.Exp

    nc = bacc.Bacc("TRN2", target_bir_lowering=False, debug=False)

    xT_d = nc.dram_tensor("xT", [S, 128, 2, T], i8, kind="ExternalInput").ap()
    xn_d = nc.dram_tensor("xn", [S, 128, NCH // 2, 2, DP], f8, kind="ExternalInput").ap()
    w_d = nc.dram_tensor("w", [128, 2, U], bf16, kind="ExternalInput").ap()
    wb_d = nc.dram_tensor("wb", [128, U // 128], f32, kind="ExternalInput").ap()
    v_d = nc.dram_tensor("v", [128, U // 128], f32, kind="ExternalInput").ap()
    # numerator + denominator per sample; the division happens on the host
    out_d = nc.dram_tensor("out", [S, D + 1], f32, kind="ExternalOutput").ap()

    with tile.TileContext(nc) as tc:
        with (
            tc.tile_pool(name="const", bufs=1) as const_pool,
            tc.tile_pool(name="xT", bufs=12) as xT_pool,
            tc.tile_pool(name="xn", bufs=8) as xn_pool,
            tc.tile_pool(name="tanh", bufs=4) as tanh_pool,
            tc.tile_pool(name="z", bufs=3) as z_pool,
            tc.tile_pool(name="wexp", bufs=4) as wexp_pool,
            tc.tile_pool(name="fin", bufs=2) as fin_pool,
            tc.tile_pool(name="score_ps", bufs=3, space="PSUM") as score_pool,
            tc.tile_pool(name="logit_ps", bufs=1, space="PSUM") as logit_pool,
            tc.tile_pool(name="c_ps", bufs=1, space="PSUM") as c_pool,
        ):
            # constants
            w_sb = const_pool.tile([128, 2, U], bf16)     # [ki, dc, u]
            nc.sync.dma_start(w_sb[:], w_d)
            v_sb = const_pool.tile([128, 2], f32)
            nc.sync.dma_start(v_sb[:], v_d)
            wb_sb = const_pool.tile([128, 2], f32)
            nc.sync.dma_start(wb_sb[:], wb_d)
            ones_sb = const_pool.tile([128, 1], bf16)
            nc.vector.memset(ones_sb[:], 1.0)
            warm_in = const_pool.tile([128, 256], bf16)
            nc.vector.memset(warm_in[:], 0.0)
            fin_all = const_pool.tile([1, S * (D + 1)], f32)

            # Single wsum accumulator: DoubleRow matmul outputs must land at
            # partition 0, and a PSUM accumulation group cannot share a bank
            # with anything else (start=True wipes the whole bank's
            # has_written bits). One group of schedule slack covers the
            # finalize-read before the next sample's first wsum.
            c0_bank = c_pool.tile([1, DP], f32)
            lg_bank = logit_pool.tile([128, 32], f32)
            lg3 = lg_bank[:].rearrange("p (a b) -> p a b", a=2)

            # HAM warmup: dummy matmuls keep the PE activity monitor busy
            # during the initial DMA fill so real GEMMs start at 2.4 GHz
            warm_ps = score_pool.tile([128, GT], f32, tag="score", name="warm")

            def emit_warm(n):
                for _ in range(n):
                    nc.tensor.matmul(
                        warm_ps[:, 0:U], warm_in[:, 0:128], warm_in[:],
                        start=True, stop=True,
                    )

            emit_warm(44)

            xT_tiles = {}       # (s, quarter) -> [128, 2, 1024] bf16
            xn_tiles = {}       # (s, half) -> [128, 16, 257] bf16
            z_tiles = {}        # g -> [128, 1024] bf16
            logit_tiles = {}    # s -> [128, 32] psum
            wexp_tiles = {}     # s -> [128, 32] bf16
            c_tiles = {}        # s -> [1, 257] psum

            def fetch_sample(s):
                """Issue the cast-DMAs for one sample (int8 -> bf16)."""
                for q in range(N_GROUPS):
                    xt = xT_pool.tile([128, 2, GT], bf16, tag="xT",
                                      name=f"xT{s}_{q}")
                    nc.gpsimd.dma_start(xt[:], xT_d[s, :, :, ts(q, GT)])
                    xT_tiles[(s, q)] = xt
                    if q == 1 or q == 3:
                        h = q // 2
                        xn = xn_pool.tile([128, NCH // 4, 2, DP], f8,
                                          tag="xn", name=f"xn{s}_{h}")
                        nc.sync.dma_start(
                            xn[:], xn_d[s, :, ts(h, NCH // 4), :, :])
                        xn_tiles[(s, h)] = xn

            def emit_l2(j, c):
                """Partition-reduce of z chunk c of group j -> logit col.

                Chunk cc = t//128 lands at logit[:, cc%2, cc//2] so the
                DoubleRow wsum pairing (ko = cc%2, cp = cc//2) lines up.
                """
                sj, gj = divmod(j, N_GROUPS)
                cc = gj * CH + c
                nc.tensor.matmul(
                    lg3[:, cc % 2, ds(cc // 2, 1)],
                    z_tiles[j][:, ts(c, 128)],
                    ones_sb[:],
                    start=True,
                    stop=True,
                )
                if c == CH - 1:
                    del z_tiles[j]

            def emit_wsum(j, c):
                """One DoubleRow chunk-pair (256 t's) of the weighted sum."""
                sj, gj = divmod(j, N_GROUPS)
                cp = gj * (CH // 2) + c
                h, cl = divmod(cp, NCH // 4)
                nc.tensor.matmul(
                    c_tiles[sj][:],
                    wexp_tiles[(sj, cp // 4)][:, :, ds(cp % 4, 1)],
                    xn_tiles[(sj, h)][:, cl, :, :],
                    start=(cp == 0),
                    stop=(cp == NCH // 2 - 1),
                    perf_mode=DR,
                )
                if cp == NCH // 2 - 1:
                    del xn_tiles[(sj, 0)], xn_tiles[(sj, 1)]

            fetch_sample(0)
            fetch_sample(1)

            for g in range(NG + LAG_W + 1):
                s, gt = divmod(g, N_GROUPS) if g < NG else (None, None)
                jl = g - LAG_L2   # group index for V-dot this iteration
                jw = g - LAG_W    # group index for wsum this iteration

                # ---- prefetch two samples ahead of the compute front ----
                if g < NG and gt == 0 and s + 2 < S:
                    fetch_sample(s + 2)

                # ---- GEMM: 2 psum tiles (uc0, uc1) of [128, 1024] ----
                if g < NG:
                    if gt == 0:
                        c_tiles[s] = c0_bank[0:1, :]
                    xt = xT_tiles[(s, gt)]
                    scs = []
                    li, n_l2 = 0, (CH if 0 <= jl < NG else 0)
                    wi, n_w = 0, (CH // 2 if 0 <= jw < NG else 0)
                    for uc in range(2):
                        sc = score_pool.tile([128, GT], f32, tag="score",
                                             name=f"sc{g}_{uc}")
                        for dc in range(2):
                            # same-weight GEMM pair back-to-back so the
                            # next LDWEIGHTS hides behind a full stream
                            for half in range(2):
                                nc.tensor.matmul(
                                    sc[:, ts(half, TT)],
                                    w_sb[:, dc, ts(uc, 128)],
                                    xt[:, dc, ts(half, TT)],
                                    start=(dc == 0),
                                    stop=(dc == 1),
                                )
                            # tail-matmul burst between GEMM pairs
                            for _ in range(2):
                                if li < n_l2:
                                    emit_l2(jl, li)
                                    li += 1
                            if wi < n_w:
                                emit_wsum(jw, wi)
                                wi += 1
                        scs.append(sc)
                    while li < n_l2:
                        emit_l2(jl, li)
                        li += 1
                    while wi < n_w:
                        emit_wsum(jw, wi)
                        wi += 1
                    del xT_tiles[(s, gt)]
                else:
                    if g == NG or g == NG + 1:
                        # keep the PE activity monitor warm through the
                        # drain so the tail matmuls run at full clock
                        emit_warm(20 if g == NG else 10)
                    for c in range(CH if 0 <= jl < NG else 0):
                        emit_l2(jl, c)
                    for c in range(CH // 2 if 0 <= jw < NG else 0):
                        emit_wsum(jw, c)

                # ---- ACT: exp per group (after its V-dots) -> fp8 ----
                if 0 <= jl < NG:
                    sj, gl = divmod(jl, N_GROUPS)
                    # [128, 2, 16] tile but only 4 cols used: the DoubleRow
                    # LDWEIGHTS ko-stride must be a multiple of 16 bytes
                    wx = wexp_pool.tile([128, 2, 16], f8, tag="wexp")
                    if jl == NG - 1:
                        # split the last exp so the final wsum pairs start
                        # as soon as the first half-group's logits land
                        for hh in range(2):
                            nc.scalar.activation(
                                wx[:, :, ts(hh, CH // 4)],
                                lg3[:, :, ds(gl * (CH // 2) + hh * (CH // 4),
                                             CH // 4)],
                                Exp)
                    else:
                        nc.scalar.activation(
                            wx[:, :, 0 : CH // 2], lg3[:, :, ts(gl, CH // 2)],
                            Exp)
                    wexp_tiles[(sj, gl)] = wx

                # ---- tanh + V-fold for this group ----
                # the last group runs at half granularity so its serial
                # tail chain (tanh->fold->V-dot->exp->wsum) is shorter
                if g < NG:
                    tanh_t = tanh_pool.tile([128, 2, GT], bf16)
                    q = z_pool.tile([128, GT], bf16, tag="q")
                    zt = z_pool.tile([128, GT], bf16, tag="z")
                    for hh in range(2 if g == NG - 1 else 1):
                        hs = ts(hh, GT // 2) if g == NG - 1 else slice(None)
                        for uc in range(2):
                            nc.scalar.activation(
                                tanh_t[:, uc, hs],
                                scs[uc][:, hs],
                                Tanh,
                                bias=wb_sb[:, ds(uc, 1)],
                            )
                        nc.vector.tensor_scalar_mul(q[:, hs],
                                                    tanh_t[:, 0, hs],
                                                    v_sb[:, ds(0, 1)])
                        nc.vector.tensor_scalar_mul(zt[:, hs],
                                                    tanh_t[:, 1, hs],
                                                    v_sb[:, ds(1, 1)])
                        nc.vector.tensor_add(zt[:, hs], zt[:, hs], q[:, hs])
                    z_tiles[g] = zt

                # ---- finalize sample after its last wsum chunk ----
                # copy num|den psum -> SBUF on the (idle) gpsimd engine;
                # the division happens host-side
                if 0 <= jw < NG and jw % N_GROUPS == N_GROUPS - 1:
                    sj = jw // N_GROUPS
                    for gl in range(N_GROUPS):
                        del wexp_tiles[(sj, gl)]
                    c_ps = c_tiles.pop(sj)
                    nc.vector.tensor_copy(
                        fin_all[0:1, ds(sj * (D + 1), D + 1)],
                        c_ps[0:1, 0 : D + 1],
                    )

            # one batched output DMA instead of 8 tiny ones
            nc.scalar.dma_start(out_d[:, :], fin_all[0:1, :])

    nc.compile()
    return nc


def _prep_inputs(inputs, W_w, W_b, V_w, V_b):
    x = np.asarray(inputs, dtype=np.float32)
    xq = np.clip(np.round(x * XS), -127, 127).astype(np.int8)     # [B, T, D]

    # xT: [B, 128(ki), 2(dc), T] with d = dc*128 + ki
    xT_full = np.ascontiguousarray(
        xq.transpose(0, 2, 1).reshape(B, 2, 128, T).transpose(0, 2, 1, 3)
    )
    # xn: fp8, [B, 128(p), 16(cp), 2(ko), 272] with t = cp*256 + ko*128 + p;
    # col 256 = 1.0 (softmax denominator), cols 257..271 zero pad so the
    # DoubleRow ko-stride is a multiple of 16 bytes
    xn_pad = np.zeros((B, T, DP), dtype=ml_dtypes.float8_e4m3)
    xn_pad[:, :, :D] = x.astype(ml_dtypes.float8_e4m3)
    xn_pad[:, :, D] = 1.0
    xn_full = np.ascontiguousarray(
        xn_pad.reshape(B, NCH // 2, 2, 128, DP).transpose(0, 3, 1, 2, 4)
    )

    w = np.ascontiguousarray(
        (np.asarray(W_w, dtype=np.float32) / XS)
        .reshape(2, 128, U)
        .transpose(1, 0, 2)
    ).astype(BF16)                                                 # [128, 2, U]
    wb = np.asarray(W_b, dtype=np.float32).reshape(U // 128, 128).T.copy()
    v = np.asarray(V_w, dtype=np.float32).reshape(U // 128, 128).T.copy()

    in_maps = []
    for c in range(N_CORES):
        sl = slice(c * S, (c + 1) * S)
        in_maps.append(
            {
                "xT": np.ascontiguousarray(xT_full[sl]),
                "xn": np.ascontiguousarray(xn_full[sl]),
                "w": w,
                "wb": wb,
                "v": v,
            }
        )
    return in_maps


def kernel(inputs, W_w, W_b, V_w, V_b):
    from concourse.bass_utils import run_bass_kernel_spmd

    if "nc" not in _CACHE:
        _CACHE["nc"] = _build()
    nc = _CACHE["nc"]

    in_maps = _prep_inputs(inputs, W_w, W_b, V_w, V_b)
    res = run_bass_kernel_spmd(nc, in_maps, core_ids=list(range(N_CORES)))
    nd = np.concatenate([r["out"] for r in res.results], axis=0)  # [B, D+1]
    out = nd[:, :D] / nd[:, D : D + 1]
    return np.asarray(out, dtype=np.float32)


# revision 66
# speedup vs baseline: 1.0020x; 1.0020x over previous
"""Trainium2 Bass kernel for additive-attention pooling.

Reference math (per sample b):
    score  = tanh(x @ W_w + W_b)          # [T, U]
    logits = score @ V_w + V_b            # [T, 1]
    attn   = softmax(logits, axis=T)
    out    = sum_t attn[t] * x[t, :]      # [D]

V_b cancels in the softmax and is dropped. Softmax max-subtraction is
skipped: logits stay in [-5, 5] for this input scale, safe in bf16 exp.

Sharding: data-parallel over batch, 8 samples per core on 8 NeuronCores.

Precision strategy (simulated end-to-end rel err 1.42e-2 vs gate 2e-2):
  - GEMM path: x ships as int8 (x*22 rounded); SWDGE (gpsimd) DMAs cast
    int8 -> bf16 in flight, landing exact small integers in SBUF. int8's
    uniform absolute quantization error beats fp8-e4m3's relative error
    ~3x for N(0,1) data, and W (the error-critical operand: its error is
    systematic across t and does not average out) stays bf16, pre-divided
    by 22 host-side so no descale is needed.
  - wsum path: xn ships as fp8-e4m3 (HWDGE, no cast) and the exp weights
    are fp8, enabling DoubleRow matmuls (2 fp8 MACs/cell) that halve the
    weighted-sum TensorE time.

Layouts (per core, S=8 samples):
  xT_i8 [S, 128, 2, T]        d = dc*128 + ki, score GEMM (d on part.)
  xn_f8 [S, 128, 16, 2, 272]  t = cp*256 + ko*128 + p; col 256 = 1.0
                              (denominator), cols 257.. pad so the
                              DoubleRow ko-stride is 16-byte aligned
  w     [128, 2, 256]         bf16 W/22, d = dc*128 + ki
  wb/v  [128, 2]              f32 per-u bias / V weights

Pipeline per group g (1024 t's, 32 groups), software-pipelined via Tile:
  1. GEMM (TensorE): per uc: 4 matmuls (2 dc x 2 halves) N=512 into a
     [128, 1024] psum tile (2 banks), W blocks stationary.
  2. tanh (ScalarE): one [128, 1024] ACT per uc, psum -> bf16 SBUF,
     bias = W_b chunk (batched ACTs amortize the 352-cyc fixed cost).
  3. V-fold (VectorE): z = V0*tanh_u0 + V1*tanh_u1, [128, 1024] ops.
  4. V-dot (TensorE): per 128-t chunk, lhsT = z chunk, rhs = ones ->
     logit col [t,1] at lg3[:, cc%2, cc//2] (the DoubleRow pairing);
     interleaved between GEMM matmuls so LDWEIGHTS stay hidden.
  5. exp (ScalarE): one [128, 2, 4] ACT per group -> fp8 weights
     (enables LAG_W=3 and a short drain tail).
  6. wsum (TensorE): DoubleRow, lhsT = fp8 weight pair [128, 2, 1],
     rhs = xn chunk-pair [128, 2, 272], accumulated over the sample in a
     single partition-0 psum row (DoubleRow outputs must start at
     partition 0, and an accumulating bank cannot be shared: start=True
     wipes the whole bank's has_written bits).
  7. finalize (VectorE): copy num|den psum row -> SBUF, DMA out; the
     division happens on the host after the gather.

HAM management: a burst of dependency-free warmup matmuls at kernel
start (during the DMA fill) and at the head of the drain keeps the PE
clock at 2.4 GHz; the last group runs tanh/fold/exp at half granularity
to shorten the serial drain chain.

Measured on 8 trn2 cores: ~111 us (baseline bf16 kernel: ~141 us).
"""

import numpy as np
import ml_dtypes

# ---- problem constants (hardcoded; kernel.py must be self-contained) ----
B, T, D, U = 64, 4096, 256, 256
N_CORES = 8
S = B // N_CORES          # samples per core
TT = 512                  # t-tile
GT = 1024                 # t's per pipeline group (2 tiles)
N_GROUPS = T // GT        # groups per sample (4)
NG = S * N_GROUPS         # total pipeline groups (32)
CH = GT // 128            # 128-row chunks per group (8)
NCH = T // 128            # chunks per sample (32)
XS = 22.0                 # int8 quantization scale for x
DP = 272                  # xn free size: D padded to a 16-byte multiple + den
LAG_L2 = 1                # V-dot lag in groups
LAG_W = 3                 # weighted-sum lag in groups (per-group exps)

BF16 = ml_dtypes.bfloat16
FP8 = ml_dtypes.float8_e4m3

_CACHE = {}


def _build():
    import concourse.bass as bass
    import concourse.tile as tile
    from concourse import bacc, mybir
    from concourse.bass import ds, ts

    f32 = mybir.dt.float32
    bf16 = mybir.dt.bfloat16
    i8 = mybir.dt.int8
    f8 = mybir.dt.float8e4
    DR = mybir.MatmulPerfMode.DoubleRow
    Tanh = mybir.ActivationFunctionType.Tanh
    Exp = mybir.ActivationFunctionType.Exp

    nc = bacc.Bacc("TRN2", target_bir_lowering=False, debug=False)

    xT_d = nc.dram_tensor("xT", [S, 128, 2, T], i8, kind="ExternalInput").ap()
    xn_d = nc.dram_tensor("xn", [S, 128, NCH // 2, 2, DP], f8, kind="ExternalInput").ap()
    w_d = nc.dram_tensor("w", [128, 2, U], bf16, kind="ExternalInput").ap()
    wb_d = nc.dram_tensor("wb", [128, U // 128], f32, kind="ExternalInput").ap()
    v_d = nc.dram_tensor("v", [128, U // 128], f32, kind="ExternalInput").ap()
    # numerator + denominator per sample; the division happens on the host
    out_d = nc.dram_tensor("out", [S, D + 1], f32, kind="ExternalOutput").ap()

    with tile.TileContext(nc) as tc:
        with (
            tc.tile_pool(name="const", bufs=1) as const_pool,
            tc.tile_pool(name="xT", bufs=16) as xT_pool,
            tc.tile_pool(name="xn", bufs=10) as xn_pool,
            tc.tile_pool(name="tanh", bufs=4) as tanh_pool,
            tc.tile_pool(name="z", bufs=3) as z_pool,
            tc.tile_pool(name="wexp", bufs=4) as wexp_pool,
            tc.tile_pool(name="fin", bufs=2) as fin_pool,
            tc.tile_pool(name="score_ps", bufs=3, space="PSUM") as score_pool,
            tc.tile_pool(name="logit_ps", bufs=1, space="PSUM") as logit_pool,
            tc.tile_pool(name="c_ps", bufs=1, space="PSUM") as c_pool,
        ):
            # constants
            w_sb = const_pool.tile([128, 2, U], bf16)     # [ki, dc, u]
            nc.sync.dma_start(w_sb[:], w_d)
            v_sb = const_pool.tile([128, 2], f32)
            nc.sync.dma_start(v_sb[:], v_d)
            wb_sb = const_pool.tile([128, 2], f32)
            nc.sync.dma_start(wb_sb[:], wb_d)
            ones_sb = const_pool.tile([128, 1], bf16)
            nc.vector.memset(ones_sb[:], 1.0)
            warm_in = const_pool.tile([128, 256], bf16)
            nc.vector.memset(warm_in[:], 0.0)
            fin_all = const_pool.tile([1, S * (D + 1)], f32)

            # Single wsum accumulator: DoubleRow matmul outputs must land at
            # partition 0, and a PSUM accumulation group cannot share a bank
            # with anything else (start=True wipes the whole bank's
            # has_written bits). One group of schedule slack covers the
            # finalize-read before the next sample's first wsum.
            c0_bank = c_pool.tile([1, DP], f32)
            lg_bank = logit_pool.tile([128, 32], f32)
            lg3 = lg_bank[:].rearrange("p (a b) -> p a b", a=2)

            # HAM warmup: dummy matmuls keep the PE activity monitor busy
            # during the initial DMA fill so real GEMMs start at 2.4 GHz
            warm_ps = score_pool.tile([128, GT], f32, tag="score", name="warm")

            def emit_warm(n):
                for _ in range(n):
                    nc.tensor.matmul(
                        warm_ps[:, 0:U], warm_in[:, 0:128], warm_in[:],
                        start=True, stop=True,
                    )

            emit_warm(44)

            xT_tiles = {}       # (s, quarter) -> [128, 2, 1024] bf16
            xn_tiles = {}       # (s, half) -> [128, 16, 257] bf16
            z_tiles = {}        # g -> [128, 1024] bf16
            logit_tiles = {}    # s -> [128, 32] psum
            wexp_tiles = {}     # s -> [128, 32] bf16
            c_tiles = {}        # s -> [1, 257] psum

            def fetch_sample(s):
                """Issue the cast-DMAs for one sample (int8 -> bf16)."""
                for q in range(N_GROUPS):
                    xt = xT_pool.tile([128, 2, GT], bf16, tag="xT",
                                      name=f"xT{s}_{q}")
                    nc.gpsimd.dma_start(xt[:], xT_d[s, :, :, ts(q, GT)])
                    xT_tiles[(s, q)] = xt
                    if q == 1 or q == 3:
                        h = q // 2
                        xn = xn_pool.tile([128, NCH // 4, 2, DP], f8,
                                          tag="xn", name=f"xn{s}_{h}")
                        nc.sync.dma_start(
                            xn[:], xn_d[s, :, ts(h, NCH // 4), :, :])
                        xn_tiles[(s, h)] = xn

            def emit_l2(j, c):
                """Partition-reduce of z chunk c of group j -> logit col.

                Chunk cc = t//128 lands at logit[:, cc%2, cc//2] so the
                DoubleRow wsum pairing (ko = cc%2, cp = cc//2) lines up.
                """
                sj, gj = divmod(j, N_GROUPS)
                cc = gj * CH + c
                nc.tensor.matmul(
                    lg3[:, cc % 2, ds(cc // 2, 1)],
                    z_tiles[j][:, ts(c, 128)],
                    ones_sb[:],
                    start=True,
                    stop=True,
                )
                if c == CH - 1:
                    del z_tiles[j]

            def emit_wsum(j, c):
                """One DoubleRow chunk-pair (256 t's) of the weighted sum."""
                sj, gj = divmod(j, N_GROUPS)
                cp = gj * (CH // 2) + c
                h, cl = divmod(cp, NCH // 4)
                nc.tensor.matmul(
                    c_tiles[sj][:],
                    wexp_tiles[(sj, cp // 4)][:, :, ds(cp % 4, 1)],
                    xn_tiles[(sj, h)][:, cl, :, :],
                    start=(cp == 0),
                    stop=(cp == NCH // 2 - 1),
                    perf_mode=DR,
                )
                if cp == NCH // 2 - 1:
                    del xn_tiles[(sj, 0)], xn_tiles[(sj, 1)]

            fetch_sample(0)
            fetch_sample(1)
            fetch_sample(2)

            for g in range(NG + LAG_W + 1):
                s, gt = divmod(g, N_GROUPS) if g < NG else (None, None)
                jl = g - LAG_L2   # group index for V-dot this iteration
                jw = g - LAG_W    # group index for wsum this iteration

                # ---- prefetch three samples ahead of the compute front ----
                if g < NG and gt == 0 and s + 3 < S:
                    fetch_sample(s + 3)

                # ---- GEMM: 2 psum tiles (uc0, uc1) of [128, 1024] ----
                if g < NG:
                    if gt == 0:
                        c_tiles[s] = c0_bank[0:1, :]
                    xt = xT_tiles[(s, gt)]
                    scs = []
                    li, n_l2 = 0, (CH if 0 <= jl < NG else 0)
                    wi, n_w = 0, (CH // 2 if 0 <= jw < NG else 0)
                    for uc in range(2):
                        sc = score_pool.tile([128, GT], f32, tag="score",
                                             name=f"sc{g}_{uc}")
                        for dc in range(2):
                            # same-weight GEMM pair back-to-back so the
                            # next LDWEIGHTS hides behind a full stream
                            for half in range(2):
                                nc.tensor.matmul(
                                    sc[:, ts(half, TT)],
                                    w_sb[:, dc, ts(uc, 128)],
                                    xt[:, dc, ts(half, TT)],
                                    start=(dc == 0),
                                    stop=(dc == 1),
                                )
                            # tail-matmul burst between GEMM pairs
                            for _ in range(2):
                                if li < n_l2:
                                    emit_l2(jl, li)
                                    li += 1
                            if wi < n_w:
                                emit_wsum(jw, wi)
                                wi += 1
                        scs.append(sc)
                    while li < n_l2:
                        emit_l2(jl, li)
                        li += 1
                    while wi < n_w:
                        emit_wsum(jw, wi)
                        wi += 1
                    del xT_tiles[(s, gt)]
                else:
                    if g == NG or g == NG + 1:
                        # keep the PE activity monitor warm through the
                        # drain so the tail matmuls run at full clock
                        emit_warm(20 if g == NG else 10)
                    for c in range(CH if 0 <= jl < NG else 0):
                        emit_l2(jl, c)
                    for c in range(CH // 2 if 0 <= jw < NG else 0):
                        emit_wsum(jw, c)

                # ---- ACT: exp per group (after its V-dots) -> fp8 ----
                if 0 <= jl < NG:
                    sj, gl = divmod(jl, N_GROUPS)
                    # [128, 2, 16] tile but only 4 cols used: the DoubleRow
                    # LDWEIGHTS ko-stride must be a multiple of 16 bytes
                    wx = wexp_pool.tile([128, 2, 16], f8, tag="wexp")
                    if jl == NG - 1:
                        # split the last exp so the final wsum pairs start
                        # as soon as the first half-group's logits land
                        for hh in range(2):
                            nc.scalar.activation(
                                wx[:, :, ts(hh, CH // 4)],
                                lg3[:, :, ds(gl * (CH // 2) + hh * (CH // 4),
                                             CH // 4)],
                                Exp)
                    else:
                        nc.scalar.activation(
                            wx[:, :, 0 : CH // 2], lg3[:, :, ts(gl, CH // 2)],
                            Exp)
                    wexp_tiles[(sj, gl)] = wx

                # ---- tanh + V-fold for this group ----
                # the last group runs at half granularity so its serial
                # tail chain (tanh->fold->V-dot->exp->wsum) is shorter
                if g < NG:
                    tanh_t = tanh_pool.tile([128, 2, GT], bf16)
                    q = z_pool.tile([128, GT], bf16, tag="q")
                    zt = z_pool.tile([128, GT], bf16, tag="z")
                    for hh in range(2 if g == NG - 1 else 1):
                        hs = ts(hh, GT // 2) if g == NG - 1 else slice(None)
                        for uc in range(2):
                            nc.scalar.activation(
                                tanh_t[:, uc, hs],
                                scs[uc][:, hs],
                                Tanh,
                                bias=wb_sb[:, ds(uc, 1)],
                            )
                        nc.vector.tensor_scalar_mul(q[:, hs],
                                                    tanh_t[:, 0, hs],
                                                    v_sb[:, ds(0, 1)])
                        nc.vector.tensor_scalar_mul(zt[:, hs],
                                                    tanh_t[:, 1, hs],
                                                    v_sb[:, ds(1, 1)])
                        nc.vector.tensor_add(zt[:, hs], zt[:, hs], q[:, hs])
                    z_tiles[g] = zt

                # ---- finalize sample after its last wsum chunk ----
                # copy num|den psum -> SBUF on the (idle) gpsimd engine;
                # the division happens host-side
                if 0 <= jw < NG and jw % N_GROUPS == N_GROUPS - 1:
                    sj = jw // N_GROUPS
                    for gl in range(N_GROUPS):
                        del wexp_tiles[(sj, gl)]
                    c_ps = c_tiles.pop(sj)
                    nc.vector.tensor_copy(
                        fin_all[0:1, ds(sj * (D + 1), D + 1)],
                        c_ps[0:1, 0 : D + 1],
                    )

            # one batched output DMA instead of 8 tiny ones
            nc.scalar.dma_start(out_d[:, :], fin_all[0:1, :])

    nc.compile()
    return nc


def _prep_inputs(inputs, W_w, W_b, V_w, V_b):
    x = np.asarray(inputs, dtype=np.float32)
    xq = np.clip(np.round(x * XS), -127, 127).astype(np.int8)     # [B, T, D]

    # xT: [B, 128(ki), 2(dc), T] with d = dc*128 + ki
    xT_full = np.ascontiguousarray(
        xq.transpose(0, 2, 1).reshape(B, 2, 128, T).transpose(0, 2, 1, 3)
    )
    # xn: fp8, [B, 128(p), 16(cp), 2(ko), 272] with t = cp*256 + ko*128 + p;
    # col 256 = 1.0 (softmax denominator), cols 257..271 zero pad so the
    # DoubleRow ko-stride is a multiple of 16 bytes
    xn_pad = np.zeros((B, T, DP), dtype=ml_dtypes.float8_e4m3)
    xn_pad[:, :, :D] = x.astype(ml_dtypes.float8_e4m3)
    xn_pad[:, :, D] = 1.0
    xn_full = np.ascontiguousarray(
        xn_pad.reshape(B, NCH // 2, 2, 128, DP).transpose(0, 3, 1, 2, 4)
    )

    w = np.ascontiguousarray(
        (np.asarray(W_w, dtype=np.float32) / XS)
        .reshape(2, 128, U)
        .transpose(1, 0, 2)
    ).astype(BF16)                                                 # [128, 2, U]
    wb = np.asarray(W_b, dtype=np.float32).reshape(U // 128, 128).T.copy()
    v = np.asarray(V_w, dtype=np.float32).reshape(U // 128, 128).T.copy()

    in_maps = []
    for c in range(N_CORES):
        sl = slice(c * S, (c + 1) * S)
        in_maps.append(
            {
                "xT": np.ascontiguousarray(xT_full[sl]),
                "xn": np.ascontiguousarray(xn_full[sl]),
                "w": w,
                "wb": wb,
                "v": v,
            }
        )
    return in_maps


def kernel(inputs, W_w, W_b, V_w, V_b):
    from concourse.bass_utils import run_bass_kernel_spmd

    if "nc" not in _CACHE:
        _CACHE["nc"] = _build()
    nc = _CACHE["nc"]

    in_maps = _prep_inputs(inputs, W_w, W_b, V_w, V_b)
    res = run_bass_kernel_spmd(nc, in_maps, core_ids=list(range(N_CORES)))
    nd = np.concatenate([r["out"] for r in res.results], axis=0)  # [B, D+1]
    out = nd[:, :D] / nd[:, D : D + 1]
    return np.asarray(out, dtype=np.float32)
